# revision 1
# baseline (speedup 1.0000x reference)
"""Trainium2 Bass kernel for sparse-in -> dense-hidden -> sampled-out net.

  val1 = relu(in_values @ W1.T[active_in_indices] + b1)        # [B, H]
  val2 = einsum('bh,bkh->bk', val1, W2[active_label_indices]) + b2[...]

Strategy: W2 is sharded row-wise across the 8 cores (per the model-parallel
sharding hint); each core STREAMS its pre-transposed shard W2T [128h, S]
through the PE against ALL 128 samples' hidden vectors, producing the full
local logit block [128 samples, S] in bf16.  The host extracts the sampled
(b, k) entries from the owning core's block and adds b2.  This replaces
per-row gathers (which serialize on the GPSIMD descriptor generator at
~4us/instruction + 3.3ns/row) with pure streaming DMA + dense matmul:
  per core: 21.5 MB in + 21.5 MB out + 84K PE columns  ->  ~150us.

stage 1 (small) stays data-parallel: 5 bucket dma_gathers of W1T rows with
the per-slot scale AND per-sample reduction folded into PE accumulation
matmuls via a host-built one-hot-scale lhsT; then relu(+b1), AllGather of
the 16 local hidden vectors across cores, transpose on PE -> V1T [h, 128].

dma_gather ucode notes (HW-verified): int16 idx wrapped [i%16, i//16],
replicated to all 8 GPSIMD-cpu partition groups; num_idxs <= 896;
<=2 SWDGE queues (3+ corrupt under multi-core concurrency).
"""

import numpy as np
import ml_dtypes

B, NNZ, F_DIM, H, C, KOUT = 128, 128, 135909, 128, 670091, 4096
N_CORES = 8
BPC = B // N_CORES          # samples per core
BUCKET = 32768              # int16-addressable rows per dma_gather
NB1 = (F_DIM + BUCKET - 1) // BUCKET   # 5 stage-1 buckets
SUB = 768                   # max num_idxs per dma_gather
NQ = 2                      # SWDGE queues (3+ corrupts under 8-core runs)
SCRATCH = 1 << 15           # SWDGE descriptor ring bytes

TC = 2048                   # streamed W2T columns per input tile
OC = 1024                   # psum/out chunk columns
MM = 512                    # matmul cols (one 2KB PSUM bank of fp32)
SHARD = (C + N_CORES - 1) // N_CORES          # 83762 rows per core
SHARD_PAD = ((SHARD + TC - 1) // TC) * TC     # 83968 (pad with zero rows)

_CACHE = {}


def _roundup(x, m):
    return (x + m - 1) // m * m


def _wrap16(ilist):
    """int16 index list -> [128, n/16] tile (wrapped, replicated x8)."""
    n = len(ilist)
    blk = np.zeros((16, n // 16), np.int16)
    blk[np.arange(n) % 16, np.arange(n) // 16] = ilist
    return np.ascontiguousarray(np.tile(blk, (8, 1)))


def build_program(nblk1, h=H, bpc=BPC, repeat=1, mode="full"):
    """nblk1: [NB1] stage-1 slot blocks per bucket.
    mode: full | stream (skip stage1+collective) | head (skip streaming)."""
    import concourse.bass as bass
    import concourse.bacc as bacc
    import concourse.mybir as mybir
    import concourse.tile as tile

    fp32 = mybir.dt.float32
    bf16 = mybir.dt.bfloat16
    i16 = mybir.dt.int16

    n1 = int(sum(nblk1)) * 128          # total stage-1 slots
    row1 = [min(BUCKET, F_DIM - b * BUCKET) for b in range(NB1)]

    nc = bacc.Bacc(
        "TRN2",
        target_bir_lowering=False,
        debug=False,
        dynamic_dma_scratch_size=SCRATCH,
        num_swdge_queues=NQ,
        num_devices=N_CORES,
    )

    w1t = nc.dram_tensor("w1t", [F_DIM, h], fp32, kind="ExternalInput")
    w2ts = nc.dram_tensor("w2ts", [h, SHARD_PAD], bf16, kind="ExternalInput")
    idx1 = nc.dram_tensor("idx1", [128, n1 // 16], i16, kind="ExternalInput")
    scl = nc.dram_tensor("scl", [128, (n1 // 128) * bpc], fp32, kind="ExternalInput")
    b1rep = nc.dram_tensor("b1rep", [bpc, h], fp32, kind="ExternalInput")
    eye = nc.dram_tensor("eye", [128, 128], bf16, kind="ExternalInput")
    out = nc.dram_tensor("logits", [B, SHARD_PAD], bf16, kind="ExternalOutput")

    with tile.TileContext(nc) as tc:
        with (
            tc.tile_pool(name="const", bufs=1) as cpool,
            tc.tile_pool(name="w2", bufs=3) as wpool,
            tc.tile_pool(name="ob", bufs=4) as opool,
            tc.tile_pool(name="psA", bufs=1, space="PSUM") as psA,
            tc.tile_pool(name="psB", bufs=2, space="PSUM") as psB,
            tc.tile_pool(name="dram", bufs=2, space="DRAM") as dpool,
        ):
            ones_col = cpool.tile([128, 1], fp32)
            nc.gpsimd.memset(ones_col[:], 1.0)
            warm_ps = psA.tile([1, 1], fp32, tag="warm")
            nc.tensor.matmul(
                warm_ps[:], lhsT=ones_col[:], rhs=ones_col[:], start=True, stop=True
            )

            idx1_t = cpool.tile([128, n1 // 16], i16)
            nc.sync.dma_start(out=idx1_t[:], in_=idx1[:, :])
            scl_t = cpool.tile([128, (n1 // 128) * bpc], fp32)
            nc.sync.dma_start(out=scl_t[:], in_=scl[:, :])
            b1_t = cpool.tile([bpc, h], fp32)
            nc.sync.dma_start(out=b1_t[:], in_=b1rep[:, :])
            eye_t = cpool.tile([128, 128], bf16)
            nc.sync.dma_start(out=eye_t[:], in_=eye[:, :])

            for _rep in range(repeat):
                if mode == "stream":
                    v1t = cpool.tile([h, B], bf16)
                    nc.gpsimd.memset(v1t[:], 0.01)
                else:
                    # ---- stage 1: hidden layer for 16 local samples ----
                    g1 = cpool.tile([128, (n1 // 128) * h], fp32)
                    qn = 0
                    off = 0
                    for b in range(NB1):
                        nb = int(nblk1[b])
                        if nb == 0:
                            continue
                        for s0 in range(0, nb, SUB // 128):
                            sb = min(SUB // 128, nb - s0)
                            o = off + s0
                            nc.gpsimd.dma_gather(
                                out_ap=g1[:, o * h : (o + sb) * h].rearrange(
                                    "p (b h) -> p b h", b=sb
                                ),
                                in_ap=w1t[b * BUCKET : b * BUCKET + row1[b], :],
                                idxs_ap=idx1_t[:, o * 8 : (o + sb) * 8],
                                num_idxs=sb * 128,
                                num_idxs_reg=sb * 128,
                                elem_size=h,
                                queue_num=qn,
                            )
                            qn = (qn + 1) % NQ
                        off += nb

                    v1_ps = psA.tile([bpc, h], fp32, tag="v1")
                    nblk_tot = n1 // 128
                    for q in range(nblk_tot):
                        nc.tensor.matmul(
                            v1_ps[:],
                            lhsT=scl_t[:, q * bpc : (q + 1) * bpc],
                            rhs=g1[:, q * h : (q + 1) * h],
                            start=(q == 0),
                            stop=(q == nblk_tot - 1),
                        )
                    v1f = cpool.tile([bpc, h], fp32)
                    nc.vector.tensor_tensor(
                        out=v1f[:], in0=v1_ps[:], in1=b1_t[:],
                        op=mybir.AluOpType.add,
                    )
                    v1r = cpool.tile([bpc, h], fp32)
                    nc.vector.tensor_scalar_max(v1r[:], v1f[:], 0.0)

                    # ---- exchange hidden vectors via DRAM AllGather ----
                    v1_bin = dpool.tile([bpc, h], fp32, tag="ci")
                    v1_bout = dpool.tile([B, h], fp32, tag="co")
                    nc.sync.dma_start(out=v1_bin[:], in_=v1r[:])
                    nc.gpsimd.collective_compute(
                        kind="AllGather",
                        op=mybir.AluOpType.bypass,
                        replica_groups=[list(range(N_CORES))],
                        ins=[v1_bin.opt()],
                        outs=[v1_bout.opt()],
                        cc_dim="Partition",
                    )
                    v1all = cpool.tile([B, h], fp32)
                    nc.sync.dma_start(out=v1all[:], in_=v1_bout[:])
                    v1b = cpool.tile([B, h], bf16)
                    nc.vector.tensor_copy(out=v1b[:], in_=v1all[:])
                    v1t_ps = psA.tile([h, B], fp32, tag="tr")
                    nc.tensor.matmul(
                        v1t_ps[:], lhsT=v1b[:], rhs=eye_t[:], start=True, stop=True
                    )
                    v1t = cpool.tile([h, B], bf16)
                    nc.scalar.copy(out=v1t[:], in_=v1t_ps[:])

                if mode == "head":
                    # touch v1t so the dependency chain completes
                    dbg = opool.tile([h, B], bf16, tag="dbg")
                    nc.vector.tensor_copy(out=dbg[:], in_=v1t[:])
                    continue

                # ---- stage 2: stream the W2T shard through the PE ----
                ci = 0
                for t0 in range(0, SHARD_PAD, TC):
                    wt = wpool.tile([128, TC], bf16, tag="w")
                    nc.scalar.dma_start(out=wt[:], in_=w2ts[:, t0 : t0 + TC])
                    for c0 in range(0, TC, OC):
                        ps = psB.tile([B, OC], fp32, tag="chunk")
                        for m0 in range(0, OC, MM):
                            nc.tensor.matmul(
                                ps[:, m0 : m0 + MM],
                                lhsT=v1t[:],
                                rhs=wt[:, c0 + m0 : c0 + m0 + MM],
                                start=True,
                                stop=True,
                            )
                        ob = opool.tile([B, OC], bf16, tag="ob")
                        nc.vector.tensor_copy(out=ob[:], in_=ps[:])
                        eng = nc.sync if ci % 2 == 0 else nc.scalar
                        eng.dma_start(
                            out=out[:, t0 + c0 : t0 + c0 + OC], in_=ob[:]
                        )
                        ci += 1
    nc.finalize()
    return nc


def make_core_inputs(in_values, active_in_indices, W1T, W2TS, b1):
    """Host-side stage-1 bucket sort + per-core inputs.

    W2TS: list of per-core [128, SHARD_PAD] bf16 shard slices.
    """
    bpc, h = BPC, H
    in_maps = []
    nblk1_g = np.zeros(NB1, np.int64)
    percore = []
    for cid in range(N_CORES):
        s = slice(cid * bpc, (cid + 1) * bpc)
        aii = active_in_indices[s]                  # [bpc, NNZ]
        b1v = (aii // BUCKET).ravel()
        order1 = np.argsort(b1v, kind="stable")
        cnt1 = np.bincount(b1v, minlength=NB1)
        percore.append((order1, cnt1))
        nblk1_g = np.maximum(nblk1_g, (cnt1 + 127) // 128)
    nblk1 = nblk1_g.astype(int)
    n1 = int(nblk1.sum()) * 128

    b1rep = np.ascontiguousarray(
        np.broadcast_to(b1.reshape(1, h), (bpc, h))
    ).astype(np.float32)
    eye = np.eye(128, dtype=ml_dtypes.bfloat16)

    for cid in range(N_CORES):
        s = slice(cid * bpc, (cid + 1) * bpc)
        aii = active_in_indices[s]
        inv = in_values[s]
        order1, cnt1 = percore[cid]

        ilist1 = np.zeros(n1, np.int16)
        scl = np.zeros((128, (n1 // 128) * bpc), np.float32)
        flat_i1 = aii.ravel()[order1]
        flat_s1 = (np.arange(bpc * NNZ) // NNZ)[order1]
        flat_v1 = inv.ravel()[order1]
        pos = 0
        src = 0
        for b in range(NB1):
            cb = int(cnt1[b])
            loc = flat_i1[src : src + cb] - b * BUCKET
            ilist1[pos : pos + cb] = loc.astype(np.int16)
            slots = pos + np.arange(cb)
            scl[slots % 128, (slots // 128) * bpc + flat_s1[src : src + cb]] = (
                flat_v1[src : src + cb]
            )
            pos += int(nblk1[b]) * 128
            src += cb

        in_maps.append(
            {
                "w1t": W1T,
                "w2ts": W2TS[cid],
                "idx1": _wrap16(ilist1),
                "scl": scl,
                "b1rep": b1rep,
                "eye": eye,
            }
        )
    return in_maps, nblk1


def make_shards(W2):
    """Pre-transposed per-core W2 shards [128, SHARD_PAD] bf16."""
    out = []
    for cid in range(N_CORES):
        lo = cid * SHARD
        hi = min(C, lo + SHARD)
        blk = np.zeros((H, SHARD_PAD), ml_dtypes.bfloat16)
        blk[:, : hi - lo] = W2[lo:hi].T.astype(ml_dtypes.bfloat16)
        out.append(np.ascontiguousarray(blk))
    return out


def postprocess(raw, active_label_indices, b2):
    """raw: list of per-core logits [B, SHARD_PAD] -> full val2 [B, KOUT]."""
    owner = active_label_indices // SHARD              # [B, KOUT]
    local = active_label_indices - owner * SHARD
    stacked = np.stack([np.asarray(r) for r in raw])   # [8, B, SHARD_PAD] bf16
    brow = np.arange(B)[:, None]
    val2 = stacked[owner, brow, local].astype(np.float32)
    return val2 + b2[active_label_indices]


def kernel(in_values, active_in_indices, active_label_indices, W1, b1, W2, b2):
    from concourse.bass_utils import run_bass_kernel_spmd

    in_values = np.asarray(in_values, dtype=np.float32)
    active_in_indices = np.asarray(active_in_indices, dtype=np.int32)
    active_label_indices = np.asarray(active_label_indices, dtype=np.int32)
    W1 = np.asarray(W1, dtype=np.float32)
    b1 = np.asarray(b1, dtype=np.float32)
    W2 = np.asarray(W2, dtype=np.float32)
    b2 = np.asarray(b2, dtype=np.float32)

    W1T = np.ascontiguousarray(W1.T)
    W2TS = make_shards(W2)
    in_maps, nblk1 = make_core_inputs(
        in_values, active_in_indices, W1T, W2TS, b1
    )
    key = tuple(nblk1)
    if _CACHE.get("key") != key:
        _CACHE["nc"] = build_program(nblk1)
        _CACHE["key"] = key
    nc = _CACHE["nc"]

    res = run_bass_kernel_spmd(nc, in_maps, list(range(N_CORES)))
    raw = [r["logits"] for r in res.results]
    val2 = postprocess(raw, active_label_indices, b2)
    return val2, active_label_indices



# revision 4
# speedup vs baseline: 21.9890x; 21.9890x over previous
"""Trainium2 Bass kernel for sparse-in -> dense-hidden -> sampled-out net.

  val1 = relu(in_values @ W1.T[active_in_indices] + b1)        # [B, H]
  val2 = einsum('bh,bkh->bk', val1, W2[active_label_indices]) + b2[...]

Strategy: W2 is sharded row-wise across the 8 cores (per the model-parallel
sharding hint); each core STREAMS its pre-transposed shard W2T [128h, S]
through the PE against ALL 128 samples' hidden vectors, producing the full
local logit block [128 samples, S] in bf16.  The host extracts the sampled
(b, k) entries from the owning core's block and adds b2.  This replaces
per-row gathers (which serialize on the GPSIMD descriptor generator at
~4us/instruction + 3.3ns/row) with pure streaming DMA + dense matmul:
  per core: 21.5 MB in + 21.5 MB out + 84K PE columns  ->  ~150us.

stage 1 (small) stays data-parallel: 5 bucket dma_gathers of W1T rows with
the per-slot scale AND per-sample reduction folded into PE accumulation
matmuls via a host-built one-hot-scale lhsT; then relu(+b1), AllGather of
the 16 local hidden vectors across cores, transpose on PE -> V1T [h, 128].

dma_gather ucode notes (HW-verified): int16 idx wrapped [i%16, i//16],
replicated to all 8 GPSIMD-cpu partition groups; num_idxs <= 896;
<=2 SWDGE queues (3+ corrupt under multi-core concurrency).
"""

import numpy as np
import ml_dtypes

B, NNZ, F_DIM, H, C, KOUT = 128, 128, 135909, 128, 670091, 4096
N_CORES = 8
BPC = B // N_CORES          # samples per core
BUCKET = 32768              # int16-addressable rows per dma_gather
NB1 = (F_DIM + BUCKET - 1) // BUCKET   # 5 stage-1 buckets
SUB = 768                   # max num_idxs per dma_gather
NQ = 2                      # SWDGE queues (3+ corrupts under 8-core runs)
SCRATCH = 1 << 15           # SWDGE descriptor ring bytes

TC = 2048                   # streamed W2T columns per input tile
OC = 1024                   # psum/out chunk columns
MM = 512                    # matmul cols (one 2KB PSUM bank of fp32)
SHARD = (C + N_CORES - 1) // N_CORES          # 83762 rows per core
SHARD_PAD = ((SHARD + TC - 1) // TC) * TC     # 83968 (pad with zero rows)

_CACHE = {}


def _roundup(x, m):
    return (x + m - 1) // m * m


def _wrap16(ilist):
    """int16 index list -> [128, n/16] tile (wrapped, replicated x8)."""
    n = len(ilist)
    blk = np.zeros((16, n // 16), np.int16)
    blk[np.arange(n) % 16, np.arange(n) // 16] = ilist
    return np.ascontiguousarray(np.tile(blk, (8, 1)))


def build_program(nblk1, h=H, bpc=BPC, repeat=1, mode="full"):
    """nblk1: [NB1] stage-1 slot blocks per bucket.
    mode: full | stream (skip stage1+collective) | head (skip streaming)."""
    import concourse.bass as bass
    import concourse.bacc as bacc
    import concourse.mybir as mybir
    import concourse.tile as tile

    fp32 = mybir.dt.float32
    bf16 = mybir.dt.bfloat16
    i16 = mybir.dt.int16

    n1 = int(sum(nblk1)) * 128          # total stage-1 slots
    row1 = [min(BUCKET, F_DIM - b * BUCKET) for b in range(NB1)]

    nc = bacc.Bacc(
        "TRN2",
        target_bir_lowering=False,
        debug=False,
        dynamic_dma_scratch_size=SCRATCH,
        num_swdge_queues=NQ,
        num_devices=N_CORES,
    )

    w1t = nc.dram_tensor("w1t", [F_DIM, h], fp32, kind="ExternalInput")
    w2ts = nc.dram_tensor("w2ts", [h, SHARD_PAD], bf16, kind="ExternalInput")
    idx1 = nc.dram_tensor("idx1", [128, n1 // 16], i16, kind="ExternalInput")
    scl = nc.dram_tensor("scl", [128, (n1 // 128) * bpc], fp32, kind="ExternalInput")
    b1rep = nc.dram_tensor("b1rep", [bpc, h], fp32, kind="ExternalInput")
    eye = nc.dram_tensor("eye", [128, 128], bf16, kind="ExternalInput")
    out = nc.dram_tensor("logits", [B, SHARD_PAD], bf16, kind="ExternalOutput")

    with tile.TileContext(nc) as tc:
        with (
            tc.tile_pool(name="const", bufs=1) as cpool,
            tc.tile_pool(name="w2", bufs=12) as wpool,
            tc.tile_pool(name="ob", bufs=8) as opool,
            tc.tile_pool(name="psA", bufs=1, space="PSUM") as psA,
            tc.tile_pool(name="psB", bufs=2, space="PSUM") as psB,
            tc.tile_pool(name="dram", bufs=2, space="DRAM") as dpool,
        ):
            ones_col = cpool.tile([128, 1], fp32)
            nc.gpsimd.memset(ones_col[:], 1.0)
            warm_ps = psA.tile([1, 1], fp32, tag="warm")
            nc.tensor.matmul(
                warm_ps[:], lhsT=ones_col[:], rhs=ones_col[:], start=True, stop=True
            )

            idx1_t = cpool.tile([128, n1 // 16], i16)
            nc.sync.dma_start(out=idx1_t[:], in_=idx1[:, :])
            scl_t = cpool.tile([128, (n1 // 128) * bpc], fp32)
            nc.sync.dma_start(out=scl_t[:], in_=scl[:, :])
            b1_t = cpool.tile([bpc, h], fp32)
            nc.sync.dma_start(out=b1_t[:], in_=b1rep[:, :])
            eye_t = cpool.tile([128, 128], bf16)
            nc.sync.dma_start(out=eye_t[:], in_=eye[:, :])

            for _rep in range(repeat):
                if mode == "stream":
                    v1t = cpool.tile([h, B], bf16)
                    nc.gpsimd.memset(v1t[:], 0.01)
                else:
                    # ---- stage 1: hidden layer for 16 local samples ----
                    g1 = cpool.tile([128, (n1 // 128) * h], fp32)
                    qn = 0
                    off = 0
                    for b in range(NB1):
                        nb = int(nblk1[b])
                        if nb == 0:
                            continue
                        for s0 in range(0, nb, SUB // 128):
                            sb = min(SUB // 128, nb - s0)
                            o = off + s0
                            nc.gpsimd.dma_gather(
                                out_ap=g1[:, o * h : (o + sb) * h].rearrange(
                                    "p (b h) -> p b h", b=sb
                                ),
                                in_ap=w1t[b * BUCKET : b * BUCKET + row1[b], :],
                                idxs_ap=idx1_t[:, o * 8 : (o + sb) * 8],
                                num_idxs=sb * 128,
                                num_idxs_reg=sb * 128,
                                elem_size=h,
                                queue_num=qn,
                            )
                            qn = (qn + 1) % NQ
                        off += nb

                    v1_ps = psA.tile([bpc, h], fp32, tag="v1")
                    nblk_tot = n1 // 128
                    for q in range(nblk_tot):
                        nc.tensor.matmul(
                            v1_ps[:],
                            lhsT=scl_t[:, q * bpc : (q + 1) * bpc],
                            rhs=g1[:, q * h : (q + 1) * h],
                            start=(q == 0),
                            stop=(q == nblk_tot - 1),
                        )
                    v1f = cpool.tile([bpc, h], fp32)
                    nc.vector.tensor_tensor(
                        out=v1f[:], in0=v1_ps[:], in1=b1_t[:],
                        op=mybir.AluOpType.add,
                    )
                    v1r = cpool.tile([bpc, h], fp32)
                    nc.vector.tensor_scalar_max(v1r[:], v1f[:], 0.0)

                    # ---- exchange hidden vectors via DRAM AllGather ----
                    v1_bin = dpool.tile([bpc, h], fp32, tag="ci")
                    v1_bout = dpool.tile([B, h], fp32, tag="co")
                    nc.sync.dma_start(out=v1_bin[:], in_=v1r[:])
                    nc.gpsimd.collective_compute(
                        kind="AllGather",
                        op=mybir.AluOpType.bypass,
                        replica_groups=[list(range(N_CORES))],
                        ins=[v1_bin.opt()],
                        outs=[v1_bout.opt()],
                        cc_dim="Partition",
                    )
                    v1all = cpool.tile([B, h], fp32)
                    nc.sync.dma_start(out=v1all[:], in_=v1_bout[:])
                    v1b = cpool.tile([B, h], bf16)
                    nc.vector.tensor_copy(out=v1b[:], in_=v1all[:])
                    v1t_ps = psA.tile([h, B], fp32, tag="tr")
                    nc.tensor.matmul(
                        v1t_ps[:], lhsT=v1b[:], rhs=eye_t[:], start=True, stop=True
                    )
                    v1t = cpool.tile([h, B], bf16)
                    nc.scalar.copy(out=v1t[:], in_=v1t_ps[:])

                if mode == "head":
                    # touch v1t so the dependency chain completes
                    dbg = opool.tile([h, B], bf16, tag="dbg")
                    nc.vector.tensor_copy(out=dbg[:], in_=v1t[:])
                    continue

                # ---- stage 2: stream the W2T shard through the PE ----
                ci = 0
                for ti, t0 in enumerate(range(0, SHARD_PAD, TC)):
                    wt = wpool.tile([128, TC], bf16, tag="w")
                    reng = nc.scalar if ti % 2 == 0 else nc.sync
                    reng.dma_start(out=wt[:], in_=w2ts[:, t0 : t0 + TC])
                    for c0 in range(0, TC, OC):
                        ps = psB.tile([B, OC], fp32, tag="chunk")
                        for m0 in range(0, OC, MM):
                            nc.tensor.matmul(
                                ps[:, m0 : m0 + MM],
                                lhsT=v1t[:],
                                rhs=wt[:, c0 + m0 : c0 + m0 + MM],
                                start=True,
                                stop=True,
                            )
                        ob = opool.tile([B, OC], bf16, tag="ob")
                        nc.vector.tensor_copy(out=ob[:], in_=ps[:])
                        eng = nc.sync if ci % 2 == 0 else nc.scalar
                        eng.dma_start(
                            out=out[:, t0 + c0 : t0 + c0 + OC], in_=ob[:]
                        )
                        ci += 1
    nc.finalize()
    return nc


def make_core_inputs(in_values, active_in_indices, W1T, W2TS, b1):
    """Host-side stage-1 bucket sort + per-core inputs.

    W2TS: list of per-core [128, SHARD_PAD] bf16 shard slices.
    """
    bpc, h = BPC, H
    in_maps = []
    nblk1_g = np.zeros(NB1, np.int64)
    percore = []
    for cid in range(N_CORES):
        s = slice(cid * bpc, (cid + 1) * bpc)
        aii = active_in_indices[s]                  # [bpc, NNZ]
        b1v = (aii // BUCKET).ravel()
        order1 = np.argsort(b1v, kind="stable")
        cnt1 = np.bincount(b1v, minlength=NB1)
        percore.append((order1, cnt1))
        nblk1_g = np.maximum(nblk1_g, (cnt1 + 127) // 128)
    nblk1 = nblk1_g.astype(int)
    n1 = int(nblk1.sum()) * 128

    b1rep = np.ascontiguousarray(
        np.broadcast_to(b1.reshape(1, h), (bpc, h))
    ).astype(np.float32)
    eye = np.eye(128, dtype=ml_dtypes.bfloat16)

    for cid in range(N_CORES):
        s = slice(cid * bpc, (cid + 1) * bpc)
        aii = active_in_indices[s]
        inv = in_values[s]
        order1, cnt1 = percore[cid]

        ilist1 = np.zeros(n1, np.int16)
        scl = np.zeros((128, (n1 // 128) * bpc), np.float32)
        flat_i1 = aii.ravel()[order1]
        flat_s1 = (np.arange(bpc * NNZ) // NNZ)[order1]
        flat_v1 = inv.ravel()[order1]
        pos = 0
        src = 0
        for b in range(NB1):
            cb = int(cnt1[b])
            loc = flat_i1[src : src + cb] - b * BUCKET
            ilist1[pos : pos + cb] = loc.astype(np.int16)
            slots = pos + np.arange(cb)
            scl[slots % 128, (slots // 128) * bpc + flat_s1[src : src + cb]] = (
                flat_v1[src : src + cb]
            )
            pos += int(nblk1[b]) * 128
            src += cb

        in_maps.append(
            {
                "w1t": W1T,
                "w2ts": W2TS[cid],
                "idx1": _wrap16(ilist1),
                "scl": scl,
                "b1rep": b1rep,
                "eye": eye,
            }
        )
    return in_maps, nblk1


def make_shards(W2):
    """Pre-transposed per-core W2 shards [128, SHARD_PAD] bf16."""
    out = []
    for cid in range(N_CORES):
        lo = cid * SHARD
        hi = min(C, lo + SHARD)
        blk = np.zeros((H, SHARD_PAD), ml_dtypes.bfloat16)
        blk[:, : hi - lo] = W2[lo:hi].T.astype(ml_dtypes.bfloat16)
        out.append(np.ascontiguousarray(blk))
    return out


def postprocess(raw, active_label_indices, b2):
    """raw: list of per-core logits [B, SHARD_PAD] -> full val2 [B, KOUT]."""
    owner = active_label_indices // SHARD              # [B, KOUT]
    local = active_label_indices - owner * SHARD
    stacked = np.stack([np.asarray(r) for r in raw])   # [8, B, SHARD_PAD] bf16
    brow = np.arange(B)[:, None]
    val2 = stacked[owner, brow, local].astype(np.float32)
    return val2 + b2[active_label_indices]


def kernel(in_values, active_in_indices, active_label_indices, W1, b1, W2, b2):
    from concourse.bass_utils import run_bass_kernel_spmd

    in_values = np.asarray(in_values, dtype=np.float32)
    active_in_indices = np.asarray(active_in_indices, dtype=np.int32)
    active_label_indices = np.asarray(active_label_indices, dtype=np.int32)
    W1 = np.asarray(W1, dtype=np.float32)
    b1 = np.asarray(b1, dtype=np.float32)
    W2 = np.asarray(W2, dtype=np.float32)
    b2 = np.asarray(b2, dtype=np.float32)

    W1T = np.ascontiguousarray(W1.T)
    W2TS = make_shards(W2)
    in_maps, nblk1 = make_core_inputs(
        in_values, active_in_indices, W1T, W2TS, b1
    )
    key = tuple(nblk1)
    if _CACHE.get("key") != key:
        _CACHE["nc"] = build_program(nblk1)
        _CACHE["key"] = key
    nc = _CACHE["nc"]

    res = run_bass_kernel_spmd(nc, in_maps, list(range(N_CORES)))
    raw = [r["logits"] for r in res.results]
    val2 = postprocess(raw, active_label_indices, b2)
    return val2, active_label_indices



# revision 5
# speedup vs baseline: 22.0401x; 1.0023x over previous
"""Trainium2 Bass kernel for sparse-in -> dense-hidden -> sampled-out net.

  val1 = relu(in_values @ W1.T[active_in_indices] + b1)        # [B, H]
  val2 = einsum('bh,bkh->bk', val1, W2[active_label_indices]) + b2[...]

Strategy: W2 is sharded row-wise across the 8 cores (per the model-parallel
sharding hint); each core STREAMS its pre-transposed shard W2T [128h, S]
through the PE against ALL 128 samples' hidden vectors, producing the full
local logit block [128 samples, S] in bf16.  The host extracts the sampled
(b, k) entries from the owning core's block and adds b2.  This replaces
per-row gathers (which serialize on the GPSIMD descriptor generator at
~4us/instruction + 3.3ns/row) with pure streaming DMA + dense matmul:
  per core: 21.5 MB in + 21.5 MB out + 84K PE columns  ->  ~150us.

stage 1 (small) stays data-parallel: 5 bucket dma_gathers of W1T rows with
the per-slot scale AND per-sample reduction folded into PE accumulation
matmuls via a host-built one-hot-scale lhsT; then relu(+b1), AllGather of
the 16 local hidden vectors across cores, transpose on PE -> V1T [h, 128].

dma_gather ucode notes (HW-verified): int16 idx wrapped [i%16, i//16],
replicated to all 8 GPSIMD-cpu partition groups; num_idxs <= 896;
<=2 SWDGE queues (3+ corrupt under multi-core concurrency).
"""

import numpy as np
import ml_dtypes

B, NNZ, F_DIM, H, C, KOUT = 128, 128, 135909, 128, 670091, 4096
N_CORES = 8
BPC = B // N_CORES          # samples per core
BUCKET = 32768              # int16-addressable rows per dma_gather
NB1 = (F_DIM + BUCKET - 1) // BUCKET   # 5 stage-1 buckets
SUB = 768                   # max num_idxs per dma_gather
NQ = 2                      # SWDGE queues (3+ corrupts under 8-core runs)
SCRATCH = 1 << 15           # SWDGE descriptor ring bytes

TC = 2048                   # streamed W2T columns per input tile
OC = 1024                   # psum/out chunk columns
MM = 512                    # matmul cols (one 2KB PSUM bank of fp32)
SHARD = (C + N_CORES - 1) // N_CORES          # 83762 rows per core
SHARD_PAD = ((SHARD + TC - 1) // TC) * TC     # 83968 (pad with zero rows)

_CACHE = {}


def _roundup(x, m):
    return (x + m - 1) // m * m


def _wrap16(ilist):
    """int16 index list -> [128, n/16] tile (wrapped, replicated x8)."""
    n = len(ilist)
    blk = np.zeros((16, n // 16), np.int16)
    blk[np.arange(n) % 16, np.arange(n) // 16] = ilist
    return np.ascontiguousarray(np.tile(blk, (8, 1)))


def build_program(nblk1, h=H, bpc=BPC, repeat=1, mode="full"):
    """nblk1: [NB1] stage-1 slot blocks per bucket.
    mode: full | stream (skip stage1+collective) | head (skip streaming)."""
    import concourse.bass as bass
    import concourse.bacc as bacc
    import concourse.mybir as mybir
    import concourse.tile as tile

    fp32 = mybir.dt.float32
    bf16 = mybir.dt.bfloat16
    i16 = mybir.dt.int16

    n1 = int(sum(nblk1)) * 128          # total stage-1 slots
    row1 = [min(BUCKET, F_DIM - b * BUCKET) for b in range(NB1)]

    nc = bacc.Bacc(
        "TRN2",
        target_bir_lowering=False,
        debug=False,
        dynamic_dma_scratch_size=SCRATCH,
        num_swdge_queues=NQ,
        num_devices=N_CORES,
    )

    w1t = nc.dram_tensor("w1t", [F_DIM, h], fp32, kind="ExternalInput")
    w2ts = nc.dram_tensor("w2ts", [h, SHARD_PAD], bf16, kind="ExternalInput")
    idx1 = nc.dram_tensor("idx1", [128, n1 // 16], i16, kind="ExternalInput")
    scl = nc.dram_tensor("scl", [128, (n1 // 128) * bpc], fp32, kind="ExternalInput")
    b1rep = nc.dram_tensor("b1rep", [bpc, h], fp32, kind="ExternalInput")
    eye = nc.dram_tensor("eye", [128, 128], bf16, kind="ExternalInput")
    out = nc.dram_tensor("logits", [B, SHARD_PAD], bf16, kind="ExternalOutput")

    with tile.TileContext(nc) as tc:
        with (
            tc.tile_pool(name="const", bufs=1) as cpool,
            tc.tile_pool(name="w2", bufs=16) as wpool,
            tc.tile_pool(name="ob", bufs=8) as opool,
            tc.tile_pool(name="psA", bufs=1, space="PSUM") as psA,
            tc.tile_pool(name="psB", bufs=2, space="PSUM") as psB,
            tc.tile_pool(name="dram", bufs=2, space="DRAM") as dpool,
        ):
            ones_col = cpool.tile([128, 1], fp32)
            nc.gpsimd.memset(ones_col[:], 1.0)
            warm_ps = psA.tile([1, 1], fp32, tag="warm")
            nc.tensor.matmul(
                warm_ps[:], lhsT=ones_col[:], rhs=ones_col[:], start=True, stop=True
            )

            idx1_t = cpool.tile([128, n1 // 16], i16)
            nc.sync.dma_start(out=idx1_t[:], in_=idx1[:, :])
            scl_t = cpool.tile([128, (n1 // 128) * bpc], fp32)
            nc.sync.dma_start(out=scl_t[:], in_=scl[:, :])
            b1_t = cpool.tile([bpc, h], fp32)
            nc.sync.dma_start(out=b1_t[:], in_=b1rep[:, :])
            eye_t = cpool.tile([128, 128], bf16)
            nc.sync.dma_start(out=eye_t[:], in_=eye[:, :])

            for _rep in range(repeat):
                if mode == "stream":
                    v1t = cpool.tile([h, B], bf16)
                    nc.gpsimd.memset(v1t[:], 0.01)
                else:
                    # ---- stage 1: hidden layer for 16 local samples ----
                    g1 = cpool.tile([128, (n1 // 128) * h], fp32)
                    qn = 0
                    off = 0
                    for b in range(NB1):
                        nb = int(nblk1[b])
                        if nb == 0:
                            continue
                        for s0 in range(0, nb, SUB // 128):
                            sb = min(SUB // 128, nb - s0)
                            o = off + s0
                            nc.gpsimd.dma_gather(
                                out_ap=g1[:, o * h : (o + sb) * h].rearrange(
                                    "p (b h) -> p b h", b=sb
                                ),
                                in_ap=w1t[b * BUCKET : b * BUCKET + row1[b], :],
                                idxs_ap=idx1_t[:, o * 8 : (o + sb) * 8],
                                num_idxs=sb * 128,
                                num_idxs_reg=sb * 128,
                                elem_size=h,
                                queue_num=qn,
                            )
                            qn = (qn + 1) % NQ
                        off += nb

                    v1_ps = psA.tile([bpc, h], fp32, tag="v1")
                    nblk_tot = n1 // 128
                    for q in range(nblk_tot):
                        nc.tensor.matmul(
                            v1_ps[:],
                            lhsT=scl_t[:, q * bpc : (q + 1) * bpc],
                            rhs=g1[:, q * h : (q + 1) * h],
                            start=(q == 0),
                            stop=(q == nblk_tot - 1),
                        )
                    v1f = cpool.tile([bpc, h], fp32)
                    nc.vector.tensor_tensor(
                        out=v1f[:], in0=v1_ps[:], in1=b1_t[:],
                        op=mybir.AluOpType.add,
                    )
                    v1r = cpool.tile([bpc, h], fp32)
                    nc.vector.tensor_scalar_max(v1r[:], v1f[:], 0.0)

                    # ---- exchange hidden vectors via DRAM AllGather ----
                    v1_bin = dpool.tile([bpc, h], fp32, tag="ci")
                    v1_bout = dpool.tile([B, h], fp32, tag="co")
                    nc.sync.dma_start(out=v1_bin[:], in_=v1r[:])
                    nc.gpsimd.collective_compute(
                        kind="AllGather",
                        op=mybir.AluOpType.bypass,
                        replica_groups=[list(range(N_CORES))],
                        ins=[v1_bin.opt()],
                        outs=[v1_bout.opt()],
                        cc_dim="Partition",
                    )
                    v1all = cpool.tile([B, h], fp32)
                    nc.sync.dma_start(out=v1all[:], in_=v1_bout[:])
                    v1b = cpool.tile([B, h], bf16)
                    nc.vector.tensor_copy(out=v1b[:], in_=v1all[:])
                    v1t_ps = psA.tile([h, B], fp32, tag="tr")
                    nc.tensor.matmul(
                        v1t_ps[:], lhsT=v1b[:], rhs=eye_t[:], start=True, stop=True
                    )
                    v1t = cpool.tile([h, B], bf16)
                    nc.scalar.copy(out=v1t[:], in_=v1t_ps[:])

                if mode == "head":
                    # touch v1t so the dependency chain completes
                    dbg = opool.tile([h, B], bf16, tag="dbg")
                    nc.vector.tensor_copy(out=dbg[:], in_=v1t[:])
                    continue

                # ---- stage 2: stream the W2T shard through the PE ----
                ci = 0
                for ti, t0 in enumerate(range(0, SHARD_PAD, TC)):
                    wt = wpool.tile([128, TC], bf16, tag="w")
                    reng = nc.scalar if ti % 2 == 0 else nc.sync
                    reng.dma_start(out=wt[:], in_=w2ts[:, t0 : t0 + TC])
                    for c0 in range(0, TC, OC):
                        ps = psB.tile([B, OC], fp32, tag="chunk")
                        for m0 in range(0, OC, MM):
                            nc.tensor.matmul(
                                ps[:, m0 : m0 + MM],
                                lhsT=v1t[:],
                                rhs=wt[:, c0 + m0 : c0 + m0 + MM],
                                start=True,
                                stop=True,
                            )
                        ob = opool.tile([B, OC], bf16, tag="ob")
                        nc.vector.tensor_copy(out=ob[:], in_=ps[:])
                        eng = nc.sync if ci % 2 == 0 else nc.scalar
                        eng.dma_start(
                            out=out[:, t0 + c0 : t0 + c0 + OC], in_=ob[:]
                        )
                        ci += 1
    nc.finalize()
    return nc


def make_core_inputs(in_values, active_in_indices, W1T, W2TS, b1):
    """Host-side stage-1 bucket sort + per-core inputs.

    W2TS: list of per-core [128, SHARD_PAD] bf16 shard slices.
    """
    bpc, h = BPC, H
    in_maps = []
    nblk1_g = np.zeros(NB1, np.int64)
    percore = []
    for cid in range(N_CORES):
        s = slice(cid * bpc, (cid + 1) * bpc)
        aii = active_in_indices[s]                  # [bpc, NNZ]
        b1v = (aii // BUCKET).ravel()
        order1 = np.argsort(b1v, kind="stable")
        cnt1 = np.bincount(b1v, minlength=NB1)
        percore.append((order1, cnt1))
        nblk1_g = np.maximum(nblk1_g, (cnt1 + 127) // 128)
    nblk1 = nblk1_g.astype(int)
    n1 = int(nblk1.sum()) * 128

    b1rep = np.ascontiguousarray(
        np.broadcast_to(b1.reshape(1, h), (bpc, h))
    ).astype(np.float32)
    eye = np.eye(128, dtype=ml_dtypes.bfloat16)

    for cid in range(N_CORES):
        s = slice(cid * bpc, (cid + 1) * bpc)
        aii = active_in_indices[s]
        inv = in_values[s]
        order1, cnt1 = percore[cid]

        ilist1 = np.zeros(n1, np.int16)
        scl = np.zeros((128, (n1 // 128) * bpc), np.float32)
        flat_i1 = aii.ravel()[order1]
        flat_s1 = (np.arange(bpc * NNZ) // NNZ)[order1]
        flat_v1 = inv.ravel()[order1]
        pos = 0
        src = 0
        for b in range(NB1):
            cb = int(cnt1[b])
            loc = flat_i1[src : src + cb] - b * BUCKET
            ilist1[pos : pos + cb] = loc.astype(np.int16)
            slots = pos + np.arange(cb)
            scl[slots % 128, (slots // 128) * bpc + flat_s1[src : src + cb]] = (
                flat_v1[src : src + cb]
            )
            pos += int(nblk1[b]) * 128
            src += cb

        in_maps.append(
            {
                "w1t": W1T,
                "w2ts": W2TS[cid],
                "idx1": _wrap16(ilist1),
                "scl": scl,
                "b1rep": b1rep,
                "eye": eye,
            }
        )
    return in_maps, nblk1


def make_shards(W2):
    """Pre-transposed per-core W2 shards [128, SHARD_PAD] bf16."""
    out = []
    for cid in range(N_CORES):
        lo = cid * SHARD
        hi = min(C, lo + SHARD)
        blk = np.zeros((H, SHARD_PAD), ml_dtypes.bfloat16)
        blk[:, : hi - lo] = W2[lo:hi].T.astype(ml_dtypes.bfloat16)
        out.append(np.ascontiguousarray(blk))
    return out


def postprocess(raw, active_label_indices, b2):
    """raw: list of per-core logits [B, SHARD_PAD] -> full val2 [B, KOUT]."""
    owner = active_label_indices // SHARD              # [B, KOUT]
    local = active_label_indices - owner * SHARD
    stacked = np.stack([np.asarray(r) for r in raw])   # [8, B, SHARD_PAD] bf16
    brow = np.arange(B)[:, None]
    val2 = stacked[owner, brow, local].astype(np.float32)
    return val2 + b2[active_label_indices]


def kernel(in_values, active_in_indices, active_label_indices, W1, b1, W2, b2):
    from concourse.bass_utils import run_bass_kernel_spmd

    in_values = np.asarray(in_values, dtype=np.float32)
    active_in_indices = np.asarray(active_in_indices, dtype=np.int32)
    active_label_indices = np.asarray(active_label_indices, dtype=np.int32)
    W1 = np.asarray(W1, dtype=np.float32)
    b1 = np.asarray(b1, dtype=np.float32)
    W2 = np.asarray(W2, dtype=np.float32)
    b2 = np.asarray(b2, dtype=np.float32)

    W1T = np.ascontiguousarray(W1.T)
    W2TS = make_shards(W2)
    in_maps, nblk1 = make_core_inputs(
        in_values, active_in_indices, W1T, W2TS, b1
    )
    key = tuple(nblk1)
    if _CACHE.get("key") != key:
        _CACHE["nc"] = build_program(nblk1)
        _CACHE["key"] = key
    nc = _CACHE["nc"]

    res = run_bass_kernel_spmd(nc, in_maps, list(range(N_CORES)))
    raw = [r["logits"] for r in res.results]
    val2 = postprocess(raw, active_label_indices, b2)
    return val2, active_label_indices

